# revision 27
# baseline (speedup 1.0000x reference)
"""Trainium2 Bass kernel for fused MultiHeadAttention + residual + LayerNorm.

Problem: B=2, L=S=2048, D=768, H=12 heads of dim 64, attention with key-padding
mask, output projection + bias, residual with q, LayerNorm(gamma, beta).

Sharding over 8 NeuronCores: data-parallel over batch (2 groups of 4 cores) x
tensor-parallel over heads (3 heads per core). Per core:
  1. project Q^T/K^T (feature-major, bf16) and V (seq-major) for its 3 heads,
  2. attention as two interleaved single-head pipelines, each owning 4 PSUM
     banks (s: 2, o: 2).  Per s-chunk: S^T matmuls -> one [128,1024] Exp
     ACTIVATE (mask folded into the per-partition bias) -> O^T accumulation
     with a ones column in V providing the softmax denominator,
  3. normalize O^T rows via reciprocal_approx_fast read straight from PSUM +
     gpsimd partition broadcast,
  4. exchange O^T head-slices with the 3 peer cores of the batch group via
     AllToAll (bf16), so each core ends with all 768 features for its own 512
     l-rows; output projection is then a local full-contraction matmul,
  5. bias + residual + LayerNorm on the own 512 rows.
Host reassembles the 8 x [4,128,768] shards into (2, 2048, 768).
"""

import sys

sys.path.insert(0, "/opt/trn_rl_repo")

import ml_dtypes
import numpy as np

import concourse.bass as bass
import concourse.tile as tile
from concourse import bacc, mybir
from concourse.bass_utils import run_bass_kernel_spmd

F32 = mybir.dt.float32
BF16 = mybir.dt.bfloat16
I32 = mybir.dt.int32

D = 768
HD = 64
HPC = 3  # heads per core
HCOLS = HPC * HD  # 192
B, L, S = 2, 2048, 2048
NCORES = 8
GROUPS = [[0, 1, 2, 3], [4, 5, 6, 7]]
KCH = D // 128  # 6 contraction chunks for projections
SCH = S // 128  # 16 s-chunks
LN_EPS = 1e-5
MASK_NEG = -1000000.0

_CACHE: dict = {}


def _build():
    nc = bacc.Bacc("TRN2", target_bir_lowering=False, debug=False, num_devices=NCORES)

    qT = nc.dram_tensor("qT", [D, L], BF16, kind="ExternalInput").ap()
    kT = nc.dram_tensor("kT", [D, S], BF16, kind="ExternalInput").ap()
    vT = nc.dram_tensor("vT", [D, S], BF16, kind="ExternalInput").ap()
    wqT = nc.dram_tensor("wqT", [D, HCOLS], BF16, kind="ExternalInput").ap()
    wkT = nc.dram_tensor("wkT", [D, HCOLS], BF16, kind="ExternalInput").ap()
    wvT = nc.dram_tensor("wvT", [D, HCOLS], BF16, kind="ExternalInput").ap()
    wtT = nc.dram_tensor("wtT", [HCOLS, D], BF16, kind="ExternalInput").ap()
    qrows = nc.dram_tensor("qrows", [4, 128, D], F32, kind="ExternalInput").ap()
    maskT = nc.dram_tensor("maskT", [128, SCH], I32, kind="ExternalInput").ap()
    bias1 = nc.dram_tensor("bias1", [1, D], F32, kind="ExternalInput").ap()
    gamma1 = nc.dram_tensor("gamma1", [1, D], F32, kind="ExternalInput").ap()
    beta1 = nc.dram_tensor("beta1", [1, D], F32, kind="ExternalInput").ap()
    out = nc.dram_tensor("out", [4, 128, D], F32, kind="ExternalOutput").ap()

    AL = mybir.AluOpType
    ACT = mybir.ActivationFunctionType

    with tile.TileContext(nc, num_cores=NCORES) as tc:
        with (
            tc.tile_pool(name="persist", bufs=1) as pp,
            tc.tile_pool(name="dram", bufs=1, space="DRAM") as dram,
        ):
            # persistent SBUF state
            QT1 = pp.tile([128, L], BF16)  # heads 0 (rows 0:64) / 1 (64:128)
            QT2 = pp.tile([128, L], BF16)  # head 2, duplicated rows
            KT1 = pp.tile([128, S], BF16)
            KT2 = pp.tile([128, S], BF16)
            V_sb = pp.tile([128, SCH, HPC, 65], BF16)
            OT12 = pp.tile([128, L], BF16)  # heads 0 (rows 0:64) / 1 (64:128)
            OT3 = pp.tile([64, L], BF16)  # head 2
            wq_sb = pp.tile([128, KCH, HCOLS], BF16)
            wk_sb = pp.tile([128, KCH, HCOLS], BF16)
            wv_sb = pp.tile([128, KCH, HCOLS], BF16)
            wt12_sb = pp.tile([128, D], BF16)
            wt3_sb = pp.tile([64, D], BF16)
            mask_i = pp.tile([128, SCH], I32)
            mask_f = pp.tile([128, SCH], F32)
            mask_bias = pp.tile([128, SCH], F32)
            gam_b = pp.tile([128, D], F32)
            bet_b = pp.tile([128, D], F32)
            bb_b = pp.tile([128, D], F32)
            eps_t = pp.tile([128, 1], F32)

            # Z-partial scratch for the output-projection ReduceScatter
            Z_dram = dram.tile([L, D], F32, name="Zd")
            Zr_dram = dram.tile([4, 128, D], F32, name="Zrd")

            # constant / weight loads
            nc.sync.dma_start(out=wq_sb, in_=wqT.rearrange("(c p) m -> p c m", p=128))
            nc.sync.dma_start(out=wk_sb, in_=wkT.rearrange("(c p) m -> p c m", p=128))
            nc.sync.dma_start(out=wv_sb, in_=wvT.rearrange("(c p) m -> p c m", p=128))
            nc.sync.dma_start(out=wt12_sb, in_=wtT[0:128, :])
            nc.sync.dma_start(out=wt3_sb, in_=wtT[128:192, :])
            nc.sync.dma_start(out=mask_i, in_=maskT[:, :])
            nc.sync.dma_start(out=gam_b, in_=gamma1.to_broadcast([128, D]))
            nc.sync.dma_start(out=bet_b, in_=beta1.to_broadcast([128, D]))
            nc.sync.dma_start(out=bb_b, in_=bias1.to_broadcast([128, D]))
            nc.vector.memset(eps_t, LN_EPS)
            ones_t = pp.tile([128, SCH, HPC, 1], BF16)
            nc.vector.memset(ones_t, 1.0)
            nc.vector.tensor_copy(V_sb[:, :, :, 64:65], ones_t)
            nc.vector.tensor_copy(mask_f, mask_i)
            # (1 - m) * MASK_NEG == m * (-MASK_NEG) + MASK_NEG
            nc.scalar.activation(
                mask_bias, mask_f, ACT.Copy, bias=float(MASK_NEG), scale=-MASK_NEG
            )

            # PE warm-up during the initial DMA window: pushes the PE HAM
            # clock-gate to 2.4 GHz before the real work.
            warm_f = pp.tile([128, 512], F32)
            nc.vector.memset(warm_f, 0.0)
            warm_l = pp.tile([128, 128], BF16)
            warm_r = pp.tile([128, 512], BF16)
            nc.vector.tensor_copy(warm_l, warm_f[:, 0:128])
            nc.vector.tensor_copy(warm_r, warm_f)
            with tc.tile_pool(name="warmps", bufs=1, space="PSUM") as wps:
                for w in range(40):
                    wp = wps.tile([128, 512], F32, tag="w", bufs=2, name=f"w{w}")
                    nc.tensor.matmul(wp, warm_l, warm_r, start=True, stop=True)

            # ---- Stage A: projections (bf16) ----
            # All 18 input chunks are prefetched upfront so the 16 DMA engines
            # saturate from t=0 instead of trickling per-projection.
            # Q/K: feature-major Q^T/K^T; heads 0+1 -> QT1/KT1 [128, L];
            # head 2 -> QT2/KT2 rows 0:64, duplicated into 64:128.
            with (
                tc.tile_pool(name="pin", bufs=1) as pin,
                tc.tile_pool(name="psp", bufs=1, space="PSUM") as psp,
            ):
                kch_t, qch_t, vch_t = [], [], []
                for xin, lst, nm in ((kT, kch_t, "k"), (qT, qch_t, "q"), (vT, vch_t, "v")):
                    for i in range(KCH):
                        ch = pin.tile([128, L], BF16, name=f"{nm}ch{i}")
                        nc.sync.dma_start(out=ch, in_=xin[128 * i : 128 * (i + 1), :])
                        lst.append(ch)
                for chunks, wsb, d1, d2 in (
                    (kch_t, wk_sb, KT1, KT2),
                    (qch_t, wq_sb, QT1, QT2),
                ):
                    for m, mp in ((0, 128), (1, 64)):
                        for n in range(4):
                            ps = psp.tile([128, 512], F32, tag="ps", bufs=3, name="ps")
                            nsl = slice(512 * n, 512 * (n + 1))
                            for i in range(KCH):
                                nc.tensor.matmul(
                                    ps[:mp],
                                    wsb[:, i, 128 * m : 128 * m + mp],
                                    chunks[i][:, nsl],
                                    start=(i == 0),
                                    stop=(i == KCH - 1),
                                )
                            if m == 0:
                                nc.any.tensor_copy(out=d1[:, nsl], in_=ps)
                            else:
                                nc.any.tensor_copy(out=d2[0:64, nsl], in_=ps[0:64])
                                nc.any.tensor_copy(out=d2[64:128, nsl], in_=ps[0:64])
                for s in range(SCH):
                    ps = psp.tile([128, 512], F32, tag="ps", bufs=3, name="psv")
                    for i in range(KCH):
                        nc.tensor.matmul(
                            ps[:, 0:HCOLS],
                            vch_t[i][:, 128 * s : 128 * (s + 1)],
                            wv_sb[:, i, :],
                            start=(i == 0),
                            stop=(i == KCH - 1),
                        )
                    nc.any.tensor_copy(
                        out=V_sb[:, s, :, 0:64],
                        in_=ps[:, 0:HCOLS].rearrange("p (h d) -> p h d", h=HPC),
                    )

            # ---- Stage B: attention, two interleaved single-head pipelines ----
            # pass = (head, 1024-wide l-block, KT/QT row-half).  Pipelines 0/1
            # run passes 2k/2k+1 on disjoint PSUM bank halves; their S^T
            # matmuls use disjoint PE row groups so they overlap.
            passes = [
                (0, 0, QT1, KT1, 0),  # head 0, block 0, rows 0:64
                (1, 0, QT1, KT1, 64),  # head 1, block 0, rows 64:128
                (2, 0, QT2, KT2, 0),  # head 2, block 0 (dup rows 0:64)
                (2, 1, QT2, KT2, 64),  # head 2, block 1 (dup rows 64:128)
                (0, 1, QT1, KT1, 0),
                (1, 1, QT1, KT1, 64),
            ]

            with (
                tc.tile_pool(name="ptp", bufs=1) as ptp,
                tc.tile_pool(name="drp", bufs=1) as drp,
                tc.tile_pool(name="zsb", bufs=3) as zsb,
                tc.tile_pool(name="aps", bufs=1, space="PSUM") as aps,
            ):
                otile = [None, None]

                def emit_st(pl, sc):
                    h, blk, QTx, KTx, r0 = passes[pl[0]]
                    l0 = 1024 * blk
                    st = aps.tile([128, 1024], F32, tag=f"s{pl[1]}", bufs=1, name=f"s{pl[1]}")
                    ssl = slice(128 * sc, 128 * (sc + 1))
                    for half in range(2):
                        nc.tensor.matmul(
                            st[:, 512 * half : 512 * (half + 1)],
                            KTx[r0 : r0 + 64, ssl],
                            QTx[r0 : r0 + 64, l0 + 512 * half : l0 + 512 * (half + 1)],
                            start=True,
                            stop=True,
                        )
                    return st

                def emit_exp(pl, sc, st):
                    p = ptp.tile([128, 1024], BF16, tag=f"p{pl[1]}", bufs=2, name=f"p{pl[1]}")
                    nc.scalar.activation(
                        p, st, ACT.Exp, bias=mask_bias[:, sc : sc + 1], scale=0.125
                    )
                    return p

                def emit_ot(pl, sc, p):
                    h = passes[pl[0]][0]
                    ot = otile[pl[1]]
                    for half in range(2):
                        nc.tensor.matmul(
                            ot[0:65, 512 * half : 512 * (half + 1)],
                            V_sb[:, sc, h, :],
                            p[:, 512 * half : 512 * (half + 1)],
                            start=(sc == 0),
                            stop=(sc == SCH - 1),
                        )

                def norm_drain(pl):
                    h, blk = passes[pl[0]][0], passes[pl[0]][1]
                    l0 = 1024 * blk
                    ot = otile[pl[1]]
                    nm = f"{pl[1]}"
                    otmp = drp.tile([65, 1024], F32, tag="ox" + nm, bufs=2, name="ox" + nm)
                    nc.vector.tensor_copy(otmp, ot[0:65, :])
                    dr = drp.tile([1, 1024], F32, tag="dr" + nm, bufs=2, name="dr" + nm)
                    nc.vector.reciprocal(dr, otmp[64:65, :])
                    rb = drp.tile([64, 1024], F32, tag="rb" + nm, bufs=2, name="rb" + nm)
                    nc.gpsimd.partition_broadcast(rb, dr)
                    dst = OT3[:, l0 : l0 + 1024] if h == 2 else OT12[
                        64 * h : 64 * (h + 1), l0 : l0 + 1024
                    ]
                    nc.vector.tensor_mul(dst, otmp[0:64, :], rb)

                def emit_z(j):
                    # Z-partial for l rows 512j..512j+512 (4 l-tiles), then
                    # ReduceScatter over the batch group.  PSUM reuses the
                    # attention s-tile tags (free between pass-pairs / in tail).
                    for lt in range(4 * j, 4 * (j + 1)):
                        tsl = slice(128 * lt, 128 * (lt + 1))
                        zp = aps.tile(
                            [128, 1024], F32, tag=f"s{lt % 2}", bufs=1, name=f"zp{lt}"
                        )
                        for n0, nw in ((0, 512), (512, 256)):
                            nc.tensor.matmul(
                                zp[:, n0 : n0 + nw],
                                OT12[:, tsl],
                                wt12_sb[:, n0 : n0 + nw],
                                start=True,
                                stop=False,
                            )
                            nc.tensor.matmul(
                                zp[:, n0 : n0 + nw],
                                OT3[:, tsl],
                                wt3_sb[:, n0 : n0 + nw],
                                start=False,
                                stop=True,
                            )
                        zb = zsb.tile([128, D], F32, tag="zb", bufs=3, name=f"zb{lt}")
                        nc.any.tensor_copy(out=zb, in_=zp[:, 0:D])
                        nc.sync.dma_start(out=Z_dram[tsl, :], in_=zb)
                    nc.gpsimd.collective_compute(
                        "ReduceScatter",
                        AL.add,
                        replica_groups=GROUPS,
                        ins=[Z_dram[512 * j : 512 * (j + 1), :].opt()],
                        outs=[Zr_dram[j].opt()],
                    )

                for pp_i in range(3):
                    pls = [(2 * pp_i, 0), (2 * pp_i + 1, 1)]
                    for pl in pls:
                        otile[pl[1]] = aps.tile(
                            [128, 1024], F32, tag=f"o{pl[1]}", bufs=1, name=f"o{pl[1]}"
                        )
                    for sc in range(SCH):
                        new_sts = [emit_st(pl, sc) for pl in pls]
                        new_ps = [emit_exp(pl, sc, new_sts[i]) for i, pl in enumerate(pls)]
                        for i, pl in enumerate(pls):
                            emit_ot(pl, sc, new_ps[i])
                    for pl in pls:
                        norm_drain(pl)
                    if pp_i == 1:
                        # O^T block 0 complete for all heads (passes 0,1,2):
                        # project + ReduceScatter rows 0:1024 under pass-pair 2.
                        emit_z(0)
                        emit_z(1)
                emit_z(2)
                emit_z(3)

            # ---- Stage C: bias + residual + LayerNorm on the own 512 rows ----
            with tc.tile_pool(name="ep", bufs=2) as ep:
                for j in range(4):
                    zr = ep.tile([128, D], F32, name="zr")
                    nc.sync.dma_start(out=zr, in_=Zr_dram[j])
                    qr = ep.tile([128, D], F32, name="qr")
                    nc.sync.dma_start(out=qr, in_=qrows[j])
                    x = ep.tile([128, D], F32, name="x")
                    nc.vector.tensor_add(x, zr, qr)
                    nc.vector.tensor_add(x, x, bb_b)
                    stats = ep.tile([128, 3, 6], F32, name="stats")
                    for g in range(3):
                        nc.vector.bn_stats(stats[:, g, :], x[:, 256 * g : 256 * (g + 1)])
                    mv = ep.tile([128, 2], F32, name="mv")
                    nc.vector.bn_aggr(mv, stats)
                    rstd = ep.tile([128, 1], F32, name="rstd")
                    nc.scalar.activation(rstd, mv[:, 1:2], ACT.Sqrt, bias=eps_t, scale=1.0)
                    nc.vector.reciprocal(rstd, rstd)
                    t1 = ep.tile([128, D], F32, name="t1")
                    nc.vector.scalar_tensor_tensor(
                        t1, x, mv[:, 0:1], gam_b, AL.subtract, AL.mult
                    )
                    o = ep.tile([128, D], F32, name="o")
                    nc.vector.scalar_tensor_tensor(
                        o, t1, rstd, bet_b, AL.mult, AL.add
                    )
                    nc.sync.dma_start(out=out[j], in_=o)

    nc.finalize()
    return nc


def _get_nc():
    if "nc" not in _CACHE:
        _CACHE["nc"] = _build()
    return _CACHE["nc"]


def build_in_maps(inputs):
    return _build_in_maps(**inputs)


def _bf16(x):
    return np.ascontiguousarray(x.astype(ml_dtypes.bfloat16))


def _build_in_maps(q, k, v, attention_mask, Wq, Wk, Wv, W, b, gamma, beta):
    q = np.asarray(q, dtype=np.float32)
    k = np.asarray(k, dtype=np.float32)
    v = np.asarray(v, dtype=np.float32)
    attention_mask = np.asarray(attention_mask, dtype=np.int32)
    Wq = np.asarray(Wq, dtype=np.float32)
    Wk = np.asarray(Wk, dtype=np.float32)
    Wv = np.asarray(Wv, dtype=np.float32)
    W = np.asarray(W, dtype=np.float32)
    b = np.asarray(b, dtype=np.float32)
    gamma = np.asarray(gamma, dtype=np.float32)
    beta = np.asarray(beta, dtype=np.float32)

    qT = [_bf16(q[i].T) for i in range(B)]
    kT = [_bf16(k[i].T) for i in range(B)]
    vT = [_bf16(v[i].T) for i in range(B)]

    maskT = [np.ascontiguousarray(attention_mask[i].reshape(SCH, 128).T) for i in range(B)]
    bias1 = np.ascontiguousarray(b.reshape(1, D))
    gamma1 = np.ascontiguousarray(gamma.reshape(1, D))
    beta1 = np.ascontiguousarray(beta.reshape(1, D))

    in_maps = []
    for c in range(NCORES):
        bi, hg = c // 4, c % 4
        cs = slice(HCOLS * hg, HCOLS * (hg + 1))
        in_maps.append(
            {
                "qT": qT[bi],
                "kT": kT[bi],
                "vT": vT[bi],
                "wqT": _bf16(Wq[cs, :].T),
                "wkT": _bf16(Wk[cs, :].T),
                "wvT": _bf16(Wv[cs, :].T),
                "wtT": _bf16(W[:, cs].T),
                "qrows": np.ascontiguousarray(
                    np.stack(
                        [
                            q[bi, 512 * j + 128 * hg : 512 * j + 128 * (hg + 1), :]
                            for j in range(4)
                        ]
                    )
                ),
                "maskT": maskT[bi],
                "bias1": bias1,
                "gamma1": gamma1,
                "beta1": beta1,
            }
        )
    return in_maps


def kernel(q, k, v, attention_mask, Wq, Wk, Wv, W, b, gamma, beta):
    nc = _get_nc()
    in_maps = _build_in_maps(q, k, v, attention_mask, Wq, Wk, Wv, W, b, gamma, beta)
    res = run_bass_kernel_spmd(nc, in_maps, core_ids=list(range(NCORES)))

    outp = np.empty((B, L, D), dtype=np.float32)
    for c in range(NCORES):
        bi, hg = c // 4, c % 4
        o = res.results[c]["out"]
        for j in range(4):
            outp[bi, 512 * j + 128 * hg : 512 * j + 128 * (hg + 1), :] = o[j]
    return outp


# revision 32
# speedup vs baseline: 1.1006x; 1.1006x over previous
"""Trainium2 Bass kernel for fused MultiHeadAttention + residual + LayerNorm.

Problem: B=2, L=S=2048, D=768, H=12 heads of dim 64, attention with key-padding
mask, output projection + bias, residual with q, LayerNorm(gamma, beta).

Sharding over 8 NeuronCores: data-parallel over batch (2 groups of 4 cores) x
tensor-parallel over heads (3 heads per core). Per core:
  1. project Q^T/K^T (feature-major, bf16) and V (seq-major) for its 3 heads,
  2. attention as two interleaved single-head pipelines, each owning 4 PSUM
     banks (s: 2, o: 2).  Per s-chunk: S^T matmuls -> one [128,1024] Exp
     ACTIVATE (mask folded into the per-partition bias) -> O^T accumulation
     with a ones column in V providing the softmax denominator,
  3. normalize O^T rows via reciprocal_approx_fast read straight from PSUM +
     gpsimd partition broadcast,
  4. exchange O^T head-slices with the 3 peer cores of the batch group via
     AllToAll (bf16), so each core ends with all 768 features for its own 512
     l-rows; output projection is then a local full-contraction matmul,
  5. bias + residual + LayerNorm on the own 512 rows.
Host reassembles the 8 x [4,128,768] shards into (2, 2048, 768).
"""

import sys

sys.path.insert(0, "/opt/trn_rl_repo")

import ml_dtypes
import numpy as np

import concourse.bass as bass
import concourse.tile as tile
from concourse import bacc, mybir
from concourse.bass_utils import run_bass_kernel_spmd

F32 = mybir.dt.float32
BF16 = mybir.dt.bfloat16
I32 = mybir.dt.int32

D = 768
HD = 64
HPC = 3  # heads per core
HCOLS = HPC * HD  # 192
B, L, S = 2, 2048, 2048
NCORES = 8
GROUPS = [[0, 1, 2, 3], [4, 5, 6, 7]]
KCH = D // 128  # 6 contraction chunks for projections
SCH = S // 128  # 16 s-chunks
LN_EPS = 1e-5
MASK_NEG = -1000000.0

_CACHE: dict = {}


def _build():
    nc = bacc.Bacc("TRN2", target_bir_lowering=False, debug=False, num_devices=NCORES)

    qT = nc.dram_tensor("qT", [D, L], BF16, kind="ExternalInput").ap()
    kT = nc.dram_tensor("kT", [D, S], BF16, kind="ExternalInput").ap()
    vT = nc.dram_tensor("vT", [D, S], BF16, kind="ExternalInput").ap()
    wqT = nc.dram_tensor("wqT", [D, HCOLS], BF16, kind="ExternalInput").ap()
    wkT = nc.dram_tensor("wkT", [D, HCOLS], BF16, kind="ExternalInput").ap()
    wvT = nc.dram_tensor("wvT", [D, HCOLS], BF16, kind="ExternalInput").ap()
    wtT = nc.dram_tensor("wtT", [HCOLS, D], BF16, kind="ExternalInput").ap()
    qrows = nc.dram_tensor("qrows", [4, 128, D], F32, kind="ExternalInput").ap()
    maskT = nc.dram_tensor("maskT", [128, SCH], I32, kind="ExternalInput").ap()
    bias1 = nc.dram_tensor("bias1", [1, D], F32, kind="ExternalInput").ap()
    gamma1 = nc.dram_tensor("gamma1", [1, D], F32, kind="ExternalInput").ap()
    beta1 = nc.dram_tensor("beta1", [1, D], F32, kind="ExternalInput").ap()
    out = nc.dram_tensor("out", [4, 128, D], F32, kind="ExternalOutput").ap()

    AL = mybir.AluOpType
    ACT = mybir.ActivationFunctionType

    with tile.TileContext(nc, num_cores=NCORES) as tc:
        with (
            tc.tile_pool(name="persist", bufs=1) as pp,
            tc.tile_pool(name="dram", bufs=1, space="DRAM") as dram,
        ):
            # persistent SBUF state
            QT1 = pp.tile([128, L], BF16)  # heads 0 (rows 0:64) / 1 (64:128)
            QT2 = pp.tile([128, L], BF16)  # head 2, duplicated rows
            KT1 = pp.tile([128, S], BF16)
            KT2 = pp.tile([128, S], BF16)
            V_sb = pp.tile([128, SCH, HPC, 65], BF16)
            OT12 = pp.tile([128, L], BF16)  # heads 0 (rows 0:64) / 1 (64:128)
            OT3 = pp.tile([64, L], BF16)  # head 2
            wq_sb = pp.tile([128, KCH, HCOLS], BF16)
            wk_sb = pp.tile([128, KCH, HCOLS], BF16)
            wv_sb = pp.tile([128, KCH, HCOLS], BF16)
            wt12_sb = pp.tile([128, D], BF16)
            wt3_sb = pp.tile([64, D], BF16)
            mask_i = pp.tile([128, SCH], I32)
            mask_f = pp.tile([128, SCH], F32)
            mask_bias = pp.tile([128, SCH], F32)
            gam_b = pp.tile([128, D], F32)
            bet_b = pp.tile([128, D], F32)
            bb_b = pp.tile([128, D], F32)
            eps_t = pp.tile([128, 1], F32)

            # Z-partial scratch for the output-projection ReduceScatter
            Z_dram = dram.tile([L, D], F32, name="Zd")
            Zr_dram = dram.tile([4, 128, D], F32, name="Zrd")

            # constant / weight loads
            nc.sync.dma_start(out=wq_sb, in_=wqT.rearrange("(c p) m -> p c m", p=128))
            nc.sync.dma_start(out=wk_sb, in_=wkT.rearrange("(c p) m -> p c m", p=128))
            nc.sync.dma_start(out=wv_sb, in_=wvT.rearrange("(c p) m -> p c m", p=128))
            nc.sync.dma_start(out=wt12_sb, in_=wtT[0:128, :])
            nc.sync.dma_start(out=wt3_sb, in_=wtT[128:192, :])
            nc.sync.dma_start(out=mask_i, in_=maskT[:, :])
            nc.sync.dma_start(out=gam_b, in_=gamma1.to_broadcast([128, D]))
            nc.sync.dma_start(out=bet_b, in_=beta1.to_broadcast([128, D]))
            nc.sync.dma_start(out=bb_b, in_=bias1.to_broadcast([128, D]))
            nc.vector.memset(eps_t, LN_EPS)
            ones_t = pp.tile([128, SCH, HPC, 1], BF16)
            nc.vector.memset(ones_t, 1.0)
            nc.vector.tensor_copy(V_sb[:, :, :, 64:65], ones_t)
            nc.vector.tensor_copy(mask_f, mask_i)
            # (1 - m) * MASK_NEG == m * (-MASK_NEG) + MASK_NEG
            nc.scalar.activation(
                mask_bias, mask_f, ACT.Copy, bias=float(MASK_NEG), scale=-MASK_NEG
            )

            # PE warm-up during the initial DMA window: pushes the PE HAM
            # clock-gate to 2.4 GHz before the real work.
            warm_f = pp.tile([128, 512], F32)
            nc.vector.memset(warm_f, 0.0)
            warm_l = pp.tile([128, 128], BF16)
            warm_r = pp.tile([128, 512], BF16)
            nc.vector.tensor_copy(warm_l, warm_f[:, 0:128])
            nc.vector.tensor_copy(warm_r, warm_f)
            with tc.tile_pool(name="warmps", bufs=1, space="PSUM") as wps:
                for w in range(40):
                    wp = wps.tile([128, 512], F32, tag="w", bufs=2, name=f"w{w}")
                    nc.tensor.matmul(wp, warm_l, warm_r, start=True, stop=True)

            # ---- Stage A: projections (bf16) ----
            # All 18 input chunks are prefetched upfront so the 16 DMA engines
            # saturate from t=0 instead of trickling per-projection.
            # Q/K: feature-major Q^T/K^T; heads 0+1 -> QT1/KT1 [128, L];
            # head 2 -> QT2/KT2 rows 0:64, duplicated into 64:128.
            with (
                tc.tile_pool(name="pin", bufs=1) as pin,
                tc.tile_pool(name="psp", bufs=1, space="PSUM") as psp,
            ):
                kch_t, qch_t, vch_t = [], [], []
                for xin, lst, nm in ((kT, kch_t, "k"), (qT, qch_t, "q"), (vT, vch_t, "v")):
                    for i in range(KCH):
                        ch = pin.tile([128, L], BF16, name=f"{nm}ch{i}")
                        nc.sync.dma_start(out=ch, in_=xin[128 * i : 128 * (i + 1), :])
                        lst.append(ch)
                # heads 0+1 (full 128-col stationary) for K then Q
                for chunks, wsb, d1 in ((kch_t, wk_sb, KT1), (qch_t, wq_sb, QT1)):
                    for n in range(4):
                        ps = psp.tile([128, 512], F32, tag="ps", bufs=3, name="ps")
                        nsl = slice(512 * n, 512 * (n + 1))
                        for i in range(KCH):
                            nc.tensor.matmul(
                                ps,
                                wsb[:, i, 0:128],
                                chunks[i][:, nsl],
                                start=(i == 0),
                                stop=(i == KCH - 1),
                            )
                        nc.any.tensor_copy(out=d1[:, nsl], in_=ps)
                # head 2 of Q and K (64-col stationaries)
                for chunks, wsb, d2 in ((kch_t, wk_sb, KT2), (qch_t, wq_sb, QT2)):
                    for n in range(4):
                        ps = psp.tile([128, 512], F32, tag="ps", bufs=3, name="ps")
                        nsl = slice(512 * n, 512 * (n + 1))
                        for i in range(KCH):
                            nc.tensor.matmul(
                                ps[0:64],
                                wsb[:, i, 128:192],
                                chunks[i][:, nsl],
                                start=(i == 0),
                                stop=(i == KCH - 1),
                            )
                        nc.any.tensor_copy(out=d2[0:64, nsl], in_=ps[0:64])
                        nc.sync.dma_start(out=d2[64:128, nsl], in_=d2[0:64, nsl])
                for s in range(SCH):
                    ps = psp.tile([128, 512], F32, tag="ps", bufs=3, name="psv")
                    for i in range(KCH):
                        nc.tensor.matmul(
                            ps[:, 0:HCOLS],
                            vch_t[i][:, 128 * s : 128 * (s + 1)],
                            wv_sb[:, i, :],
                            start=(i == 0),
                            stop=(i == KCH - 1),
                        )
                    nc.any.tensor_copy(
                        out=V_sb[:, s, :, 0:64],
                        in_=ps[:, 0:HCOLS].rearrange("p (h d) -> p h d", h=HPC),
                    )

            # ---- Stage B: attention, two interleaved single-head pipelines ----
            # pass = (head, 1024-wide l-block, KT/QT row-half).  Pipelines 0/1
            # run passes 2k/2k+1 on disjoint PSUM bank halves; their S^T
            # matmuls use disjoint PE row groups so they overlap.
            passes = [
                (0, 0, QT1, KT1, 0),  # head 0, block 0, rows 0:64
                (1, 0, QT1, KT1, 64),  # head 1, block 0, rows 64:128
                (2, 0, QT2, KT2, 0),  # head 2, block 0 (dup rows 0:64)
                (2, 1, QT2, KT2, 64),  # head 2, block 1 (dup rows 64:128)
                (0, 1, QT1, KT1, 0),
                (1, 1, QT1, KT1, 64),
            ]

            with (
                tc.tile_pool(name="ptp", bufs=1) as ptp,
                tc.tile_pool(name="drp", bufs=1) as drp,
                tc.tile_pool(name="zsb", bufs=3) as zsb,
                tc.tile_pool(name="aps", bufs=1, space="PSUM") as aps,
            ):
                otile = [None, None]

                def alloc_st(pl):
                    return aps.tile(
                        [128, 1024], F32, tag=f"s{pl[1]}", bufs=1, name=f"s{pl[1]}"
                    )

                def emit_st_half(pl, sc, st, half):
                    h, blk, QTx, KTx, r0 = passes[pl[0]]
                    l0 = 1024 * blk
                    ssl = slice(128 * sc, 128 * (sc + 1))
                    nc.tensor.matmul(
                        st[:, 512 * half : 512 * (half + 1)],
                        KTx[r0 : r0 + 64, ssl],
                        QTx[r0 : r0 + 64, l0 + 512 * half : l0 + 512 * (half + 1)],
                        start=True,
                        stop=True,
                    )

                def emit_exp(pl, sc, st):
                    p = ptp.tile([128, 1024], BF16, tag=f"p{pl[1]}", bufs=2, name=f"p{pl[1]}")
                    nc.scalar.activation(
                        p, st, ACT.Exp, bias=mask_bias[:, sc : sc + 1], scale=0.125
                    )
                    return p

                def emit_ot(pl, sc, p):
                    h = passes[pl[0]][0]
                    ot = otile[pl[1]]
                    for half in range(2):
                        nc.tensor.matmul(
                            ot[0:65, 512 * half : 512 * (half + 1)],
                            V_sb[:, sc, h, :],
                            p[:, 512 * half : 512 * (half + 1)],
                            start=(sc == 0),
                            stop=(sc == SCH - 1),
                        )

                def norm_drain(pl):
                    h, blk = passes[pl[0]][0], passes[pl[0]][1]
                    l0 = 1024 * blk
                    ot = otile[pl[1]]
                    nm = f"{pl[1]}"
                    otmp = drp.tile([65, 1024], F32, tag="ox" + nm, bufs=2, name="ox" + nm)
                    nc.vector.tensor_copy(otmp, ot[0:65, :])
                    dr = drp.tile([1, 1024], F32, tag="dr" + nm, bufs=2, name="dr" + nm)
                    nc.vector.reciprocal(dr, otmp[64:65, :])
                    rb = drp.tile([64, 1024], F32, tag="rb" + nm, bufs=2, name="rb" + nm)
                    nc.gpsimd.partition_broadcast(rb, dr)
                    dst = OT3[:, l0 : l0 + 1024] if h == 2 else OT12[
                        64 * h : 64 * (h + 1), l0 : l0 + 1024
                    ]
                    nc.vector.tensor_mul(dst, otmp[0:64, :], rb)

                def emit_z(j):
                    # Z-partial for l rows 512j..512j+512 (4 l-tiles), then
                    # ReduceScatter over the batch group.  PSUM reuses the
                    # attention s-tile tags (free between pass-pairs / in tail).
                    for lt in range(4 * j, 4 * (j + 1)):
                        tsl = slice(128 * lt, 128 * (lt + 1))
                        zp = aps.tile(
                            [128, 1024], F32, tag=f"s{lt % 2}", bufs=1, name=f"zp{lt}"
                        )
                        for n0, nw in ((0, 512), (512, 256)):
                            nc.tensor.matmul(
                                zp[:, n0 : n0 + nw],
                                OT12[:, tsl],
                                wt12_sb[:, n0 : n0 + nw],
                                start=True,
                                stop=False,
                            )
                            nc.tensor.matmul(
                                zp[:, n0 : n0 + nw],
                                OT3[:, tsl],
                                wt3_sb[:, n0 : n0 + nw],
                                start=False,
                                stop=True,
                            )
                        zb = zsb.tile([128, D], F32, tag="zb", bufs=3, name=f"zb{lt}")
                        nc.any.tensor_copy(out=zb, in_=zp[:, 0:D])
                        nc.sync.dma_start(out=Z_dram[tsl, :], in_=zb)
                    nc.gpsimd.collective_compute(
                        "ReduceScatter",
                        AL.add,
                        replica_groups=GROUPS,
                        ins=[Z_dram[512 * j : 512 * (j + 1), :].opt()],
                        outs=[Zr_dram[j].opt()],
                    )

                for pp_i in range(3):
                    pls = [(2 * pp_i, 0), (2 * pp_i + 1, 1)]
                    for pl in pls:
                        otile[pl[1]] = aps.tile(
                            [128, 1024], F32, tag=f"o{pl[1]}", bufs=1, name=f"o{pl[1]}"
                        )
                    for sc in range(SCH):
                        # same-half S^T matmuls of the two pipelines are
                        # adjacent so their disjoint PE row groups overlap
                        new_sts = [alloc_st(pl) for pl in pls]
                        for half in range(2):
                            for i, pl in enumerate(pls):
                                emit_st_half(pl, sc, new_sts[i], half)
                        new_ps = [emit_exp(pl, sc, new_sts[i]) for i, pl in enumerate(pls)]
                        for i, pl in enumerate(pls):
                            emit_ot(pl, sc, new_ps[i])
                    for pl in pls:
                        norm_drain(pl)
                    if pp_i == 1:
                        # O^T block 0 complete for all heads (passes 0,1,2):
                        # project + ReduceScatter rows 0:1024 under pass-pair 2.
                        emit_z(0)
                        emit_z(1)
                emit_z(2)
                emit_z(3)

            # ---- Stage C: bias + residual + LayerNorm on the own 512 rows ----
            with tc.tile_pool(name="ep", bufs=2) as ep:
                for j in range(4):
                    zr = ep.tile([128, D], F32, name="zr")
                    nc.sync.dma_start(out=zr, in_=Zr_dram[j])
                    qr = ep.tile([128, D], F32, name="qr")
                    nc.sync.dma_start(out=qr, in_=qrows[j])
                    x = ep.tile([128, D], F32, name="x")
                    nc.vector.tensor_add(x, zr, qr)
                    nc.vector.tensor_add(x, x, bb_b)
                    stats = ep.tile([128, 3, 6], F32, name="stats")
                    for g in range(3):
                        nc.vector.bn_stats(stats[:, g, :], x[:, 256 * g : 256 * (g + 1)])
                    mv = ep.tile([128, 2], F32, name="mv")
                    nc.vector.bn_aggr(mv, stats)
                    rstd = ep.tile([128, 1], F32, name="rstd")
                    nc.scalar.activation(rstd, mv[:, 1:2], ACT.Sqrt, bias=eps_t, scale=1.0)
                    nc.vector.reciprocal(rstd, rstd)
                    t1 = ep.tile([128, D], F32, name="t1")
                    nc.vector.scalar_tensor_tensor(
                        t1, x, mv[:, 0:1], gam_b, AL.subtract, AL.mult
                    )
                    o = ep.tile([128, D], F32, name="o")
                    nc.vector.scalar_tensor_tensor(
                        o, t1, rstd, bet_b, AL.mult, AL.add
                    )
                    nc.sync.dma_start(out=out[j], in_=o)

    nc.finalize()
    return nc


def _get_nc():
    if "nc" not in _CACHE:
        _CACHE["nc"] = _build()
    return _CACHE["nc"]


def build_in_maps(inputs):
    return _build_in_maps(**inputs)


def _bf16(x):
    return np.ascontiguousarray(x.astype(ml_dtypes.bfloat16))


def _build_in_maps(q, k, v, attention_mask, Wq, Wk, Wv, W, b, gamma, beta):
    q = np.asarray(q, dtype=np.float32)
    k = np.asarray(k, dtype=np.float32)
    v = np.asarray(v, dtype=np.float32)
    attention_mask = np.asarray(attention_mask, dtype=np.int32)
    Wq = np.asarray(Wq, dtype=np.float32)
    Wk = np.asarray(Wk, dtype=np.float32)
    Wv = np.asarray(Wv, dtype=np.float32)
    W = np.asarray(W, dtype=np.float32)
    b = np.asarray(b, dtype=np.float32)
    gamma = np.asarray(gamma, dtype=np.float32)
    beta = np.asarray(beta, dtype=np.float32)

    qT = [_bf16(q[i].T) for i in range(B)]
    kT = [_bf16(k[i].T) for i in range(B)]
    vT = [_bf16(v[i].T) for i in range(B)]

    maskT = [np.ascontiguousarray(attention_mask[i].reshape(SCH, 128).T) for i in range(B)]
    bias1 = np.ascontiguousarray(b.reshape(1, D))
    gamma1 = np.ascontiguousarray(gamma.reshape(1, D))
    beta1 = np.ascontiguousarray(beta.reshape(1, D))

    in_maps = []
    for c in range(NCORES):
        bi, hg = c // 4, c % 4
        cs = slice(HCOLS * hg, HCOLS * (hg + 1))
        in_maps.append(
            {
                "qT": qT[bi],
                "kT": kT[bi],
                "vT": vT[bi],
                "wqT": _bf16(Wq[cs, :].T),
                "wkT": _bf16(Wk[cs, :].T),
                "wvT": _bf16(Wv[cs, :].T),
                "wtT": _bf16(W[:, cs].T),
                "qrows": np.ascontiguousarray(
                    np.stack(
                        [
                            q[bi, 512 * j + 128 * hg : 512 * j + 128 * (hg + 1), :]
                            for j in range(4)
                        ]
                    )
                ),
                "maskT": maskT[bi],
                "bias1": bias1,
                "gamma1": gamma1,
                "beta1": beta1,
            }
        )
    return in_maps


def kernel(q, k, v, attention_mask, Wq, Wk, Wv, W, b, gamma, beta):
    nc = _get_nc()
    in_maps = _build_in_maps(q, k, v, attention_mask, Wq, Wk, Wv, W, b, gamma, beta)
    res = run_bass_kernel_spmd(nc, in_maps, core_ids=list(range(NCORES)))

    outp = np.empty((B, L, D), dtype=np.float32)
    for c in range(NCORES):
        bi, hg = c // 4, c % 4
        o = res.results[c]["out"]
        for j in range(4):
            outp[bi, 512 * j + 128 * hg : 512 * j + 128 * (hg + 1), :] = o[j]
    return outp
